# revision 4
# baseline (speedup 1.0000x reference)
"""Sparse masked attention on 8 TRN2 NeuronCores.

reference:  O = softmax((Q K^T * mq[:,None] + log(mk[None,:])) / 8) @ V
  - rows with mq=0: scores all equal -> uniform average of V over mk=1 keys
  - keys with mk=0: exactly dropped from the softmax

Strategy: batch (b=8) is data-parallel across the 8 cores. On the host we
compact each batch to its unmasked queries/keys (~n/2 each), so each core
computes a dense ~2176x2176 attention instead of 4096x4096:

  S^T[mchunk, n] = (Kc^T chunk).T @ Qc^T      (TensorE bf16, d=64, row-group
                                               pairs: two chunks co-stream)
  P^T = exp(S^T / 8) -> bf16                  (ScalarE exact / VectorE fast-exp,
                                               one instruction per CHUNK PAIR:
                                               both chunks' scores live in one
                                               [128,1024] 2-bank PSUM tile)
  O^T[65, n]    += Vext[mchunk].T @ P^T       (TensorE bf16, k=128, single
                                               PSUM accumulator per n-block)

where Vext = [V | 1]: the ones column accumulates the softmax denominator.
No row-max subtraction is needed: scores/8 ~ N(0,1), exp stays in range.
n-blocks are [512 x 4, remainder] so the pair-exp instructions carry no
padding waste. The host divides by the denominator, scatters rows back,
and fills masked query rows with mean(V[mk=1]).
"""

import numpy as np
import ml_dtypes

N_CORES = 8
W = 512  # n-block width (PSUM bank / fp32-accum matmul free-dim limit)


_build_cache = {}


def _widths(ncap):
    ws = [W] * (ncap // W)
    if ncap % W:
        ws.append(ncap % W)
    return ws


def _build(ncap, mcap):
    """Per-core graph. Inputs (per core):
      qt   [64, ncap]            bf16   Q^T (compacted, padded)
      ktp  [128, npairs*128]     bf16   K^T chunk pairs: pair p = chunk 2p on
                                        partitions 0-63, chunk 2p+1 on 64-127
      vext [128, mchunks*65]     bf16   partition-major Vext chunks: partition
                                        r, cols [c*65:(c+1)*65] = Vext row
                                        c*128+r = [V row | 1.0] (0 if padding)
    Output: out [nblocks*65, W] f32: block j rows [65j:65j+65], cols [:w_j] =
      [ O^T numerator (64 rows) ; denominator (1 row) ] for n-cols j*W..+w_j.
    """
    key = (ncap, mcap)
    if key in _build_cache:
        return _build_cache[key]

    import concourse.bacc as bacc
    import concourse.mybir as mybir
    import concourse.tile as tile

    f32 = mybir.dt.float32
    bf16 = mybir.dt.bfloat16
    i16 = mybir.dt.int16
    mchunks = mcap // 128
    npairs = (mchunks + 1) // 2
    odd = mchunks % 2 == 1
    widths = _widths(ncap)
    blocks, off = [], 0
    for wd in widths:
        blocks.append((off, wd))
        off += wd
    nblocks = len(blocks)
    EXP = mybir.ActivationFunctionType.Exp

    # Schraudolph fast exp on bf16 bit pattern, via int16:
    #   i16 = (int16)(s * (2^7/ln2)/8 + (127*2^7 - C)) ; bitcast -> bf16
    FEXP_A = float(2 ** 7 / np.log(2.0) / 8.0)
    FEXP_B = float(127 * 2 ** 7 - 7.5)

    nc = bacc.Bacc("TRN2", target_bir_lowering=False, debug=False,
                   num_devices=N_CORES)
    qt_d = nc.dram_tensor("qt", [64, ncap], bf16, kind="ExternalInput")
    ktp_d = nc.dram_tensor("ktp", [128, npairs * 128], bf16,
                           kind="ExternalInput")
    vext_d = nc.dram_tensor("vext", [128, mchunks * 65], bf16,
                            kind="ExternalInput")
    out_d = nc.dram_tensor("out", [nblocks * 65, W], f32,
                           kind="ExternalOutput")

    with tile.TileContext(nc) as tc:
        with (
            tc.tile_pool(name="resident", bufs=1) as resident,
            tc.tile_pool(name="pt", bufs=5) as ptp,
            tc.tile_pool(name="osb", bufs=2) as osbp,
            tc.tile_pool(name="psum_st", bufs=3, space="PSUM") as psum_st,
            tc.tile_pool(name="psum_o", bufs=2, space="PSUM") as psum_o,
        ):
            # input DMAs: small head slices first so compute starts early;
            # configs spread over idle sequencers so they don't serialize
            kt_sb = resident.tile([128, npairs * 128], bf16)
            kcut = min(384, npairs * 128)
            nc.sync.dma_start(kt_sb[:, 0:kcut], ktp_d[:, 0:kcut])
            w0 = blocks[0][1]
            qt_sb = resident.tile([128, ncap], bf16)
            nc.gpsimd.dma_start(qt_sb[0:64, 0:w0], qt_d[:, 0:w0])
            nc.gpsimd.dma_start(qt_sb[64:128, 0:w0], qt_d[:, 0:w0])
            v_sb = resident.tile([128, mchunks * 65], bf16)
            vcut = min(130, mchunks * 65)
            nc.sync.dma_start(v_sb[:, 0:vcut], vext_d[:, 0:vcut])
            if kcut < npairs * 128:
                nc.gpsimd.dma_start(kt_sb[:, kcut:], ktp_d[:, kcut:])
            if vcut < mchunks * 65:
                nc.sync.dma_start(v_sb[:, vcut:], vext_d[:, vcut:])
            if ncap > w0:
                nc.gpsimd.dma_start(qt_sb[0:64, w0:], qt_d[:, w0:])
                nc.gpsimd.dma_start(qt_sb[64:128, w0:], qt_d[:, w0:])

            pending_drain = None  # (jb, olo, w) of the previous block

            def drain(jb, olo, w):
                osb = osbp.tile([65, w], f32, tag="osb")
                nc.scalar.copy(osb[:, :], olo[:, :])
                nc.gpsimd.dma_start(out_d[jb * 65:(jb + 1) * 65, 0:w],
                                    osb[:, :])

            # flat pipeline over (block, pair-slot); PV for slot s is issued
            # while slot s+1's S matmuls queue, so the PE always has
            # dependency-free work ahead of the exp-gated PV matmuls
            slots = [(jb, j0, w, p) for jb, (j0, w) in enumerate(blocks)
                     for p in range(npairs)]
            olos = {}
            prev = None  # (jb, w, p, pt_tile, is_pair)

            def pv(jb, w, p, pt, is_pair):
                olo = olos[jb]
                mi = 2 * p
                nc.tensor.matmul(
                    olo[:, :], v_sb[:, mi * 65:(mi + 1) * 65],
                    pt[:, 0:w].bitcast(bf16) if pt.dtype == i16
                    else pt[:, 0:w],
                    start=(mi == 0), stop=(not is_pair and mi == mchunks - 1),
                    tile_position=(0, 0), skip_group_check=True)
                if is_pair:
                    nc.tensor.matmul(
                        olo[:, :], v_sb[:, (mi + 1) * 65:(mi + 2) * 65],
                        pt[:, 512:512 + w].bitcast(bf16) if pt.dtype == i16
                        else pt[:, 512:512 + w],
                        start=False, stop=(mi + 1 == mchunks - 1),
                        tile_position=(0, 0), skip_group_check=True)

            for jb, j0, w, p in slots:
                if p == 0:
                    olo = psum_o.tile([65, w], f32, tag="olo")
                    olos[jb] = olo
                is_pair = (2 * p + 1 < mchunks)
                # S^T for the two chunks of pair p: k=64 row groups 0-63 /
                # 64-127 co-stream on the PE; outputs land in the two PSUM
                # banks of one [128, 1024] tile so one exp instruction can
                # cover both chunks
                st = psum_st.tile([128, 1024], f32, tag="st")
                nc.tensor.matmul(
                    st[:, 0:w], kt_sb[0:64, p * 128:(p + 1) * 128],
                    qt_sb[0:64, j0:j0 + w],
                    start=True, stop=True, tile_position=(0, 0),
                    skip_group_check=True)
                if is_pair:
                    nc.tensor.matmul(
                        st[:, 512:512 + w], kt_sb[64:128, p * 128:(p + 1) * 128],
                        qt_sb[64:128, j0:j0 + w],
                        start=True, stop=True, tile_position=(64, 0),
                        skip_group_check=True)
                # previous slot's PV matmuls go here so the PE has work
                # queued ahead of the exp-dependent ones
                if prev is not None:
                    pv(*prev)
                # P^T = exp(S^T/8): exact on ScalarE (even slots), fast-exp
                # on VectorE (odd slots); full-width slots use ONE
                # instruction spanning both 512-col chunks
                on_act = (p % 2 == 0) if is_pair else True
                hi = 512 + w if is_pair else w
                if on_act:
                    pt = ptp.tile([128, 1024], bf16, tag="pt")
                    if w == 512 or not is_pair:
                        nc.scalar.activation(pt[:, 0:hi], st[:, 0:hi], EXP,
                                             scale=0.125)
                    else:
                        nc.scalar.activation(pt[:, 0:w], st[:, 0:w], EXP,
                                             scale=0.125)
                        nc.scalar.activation(pt[:, 512:hi], st[:, 512:hi],
                                             EXP, scale=0.125)
                else:
                    pt = ptp.tile([128, 1024], i16, tag="pt")
                    if w == 512:
                        nc.vector.tensor_scalar(
                            pt[:, 0:hi], st[:, 0:hi], FEXP_A, FEXP_B,
                            mybir.AluOpType.mult, mybir.AluOpType.add)
                    else:
                        nc.vector.tensor_scalar(
                            pt[:, 0:w], st[:, 0:w], FEXP_A, FEXP_B,
                            mybir.AluOpType.mult, mybir.AluOpType.add)
                        nc.vector.tensor_scalar(
                            pt[:, 512:hi], st[:, 512:hi], FEXP_A, FEXP_B,
                            mybir.AluOpType.mult, mybir.AluOpType.add)
                prev = (jb, w, p, pt, is_pair)
                # previous block's output drain, deferred here so it does
                # not convoy this block's exps in the ACT FIFO
                if p == 1 and pending_drain is not None:
                    drain(*pending_drain)
                    pending_drain = None
                if p == npairs - 1:
                    pending_drain = (jb, olos[jb], w)
            pv(*prev)
            drain(*pending_drain)

    nc.compile()
    _build_cache[key] = nc
    return nc


def _run(inputs, trace=False):
    queries = np.asarray(inputs["queries"], dtype=np.float32)
    keys = np.asarray(inputs["keys"], dtype=np.float32)
    values = np.asarray(inputs["values"], dtype=np.float32)
    mask_query = np.asarray(inputs["mask_query"])
    mask_key = np.asarray(inputs["mask_key"])

    b, n, d = queries.shape
    dv = values.shape[2]
    assert b == N_CORES, f"batch {b} != {N_CORES} cores"
    bf = ml_dtypes.bfloat16

    idx_q = [np.flatnonzero(mask_query[i]) for i in range(b)]
    idx_k = [np.flatnonzero(mask_key[i]) for i in range(b)]
    ncap = max(max(len(ix) for ix in idx_q), 64)
    mcap = ((max(max(len(ix) for ix in idx_k), 1) + 127) // 128) * 128
    mchunks = mcap // 128
    npairs = (mchunks + 1) // 2
    bwidths = _widths(ncap)
    nblocks = len(bwidths)

    qt = np.zeros((b, 64, ncap), bf)
    ktp = np.zeros((b, 128, npairs * 128), bf)
    vext = np.zeros((b, 128, mchunks * 65), bf)
    for i in range(b):
        nq, nk = len(idx_q[i]), len(idx_k[i])
        qt[i, :, :nq] = queries[i, idx_q[i]].T.astype(bf)
        kc_t = np.zeros((64, mcap), np.float32)
        kc_t[:, :nk] = keys[i, idx_k[i]].T
        kc_t = kc_t.astype(bf)
        for p in range(npairs):
            ktp[i, 0:64, p * 128:(p + 1) * 128] = \
                kc_t[:, (2 * p) * 128:(2 * p + 1) * 128]
            if 2 * p + 1 < mchunks:
                ktp[i, 64:128, p * 128:(p + 1) * 128] = \
                    kc_t[:, (2 * p + 1) * 128:(2 * p + 2) * 128]
        ve = np.zeros((mcap, 65), np.float32)
        ve[:nk, :dv] = values[i, idx_k[i]]
        ve[:nk, dv] = 1.0
        # partition-major: [chunk, row] -> [row_in_chunk, chunk*65+col]
        vext[i] = ve.reshape(mchunks, 128, 65).transpose(1, 0, 2) \
                    .reshape(128, mchunks * 65).astype(bf)

    nc = _build(ncap, mcap)

    from concourse.bass_utils import run_bass_kernel_spmd
    in_maps = [{"qt": qt[i], "ktp": ktp[i], "vext": vext[i]} for i in range(b)]
    res = run_bass_kernel_spmd(nc, in_maps, core_ids=list(range(N_CORES)),
                               trace=trace)

    out = np.empty((b, n, dv), np.float32)
    for i in range(b):
        ot = res.results[i]["out"]  # [nblocks*65, W]
        nq, nk = len(idx_q[i]), len(idx_k[i])
        full = np.concatenate(
            [ot[jb * 65:(jb + 1) * 65, :bwidths[jb]]
             for jb in range(nblocks)], axis=1)
        num = full[:dv, :nq]
        den = full[dv, :nq]
        if nk > 0:
            out[i, :, :] = values[i, idx_k[i]].mean(axis=0)
        else:
            out[i, :, :] = 0.0
        if nq > 0:
            out[i, idx_q[i], :] = (num / den).T
    return out, res


def kernel(**inputs):
    out, _ = _run(inputs, trace=False)
    return out
